# revision 4
# baseline (speedup 1.0000x reference)
"""Trainium2 Bass kernel: 3D factorized-position attention (dense_transformer).

Reference computation (per batch b of 8):
    x = fmap[b].reshape(256, 1568)                       # channels x positions
    qkv = W_qkv @ x ; q,k,v heads of dim 128, 4 heads
    emb[n,128] = pos_f+pos_h+pos_w broadcast-sum
    sim = (q*scale) @ k^T + (q*scale) @ emb^T  ==  qs @ (k+embT)^T
    out = softmax(sim) @ v, reassembled to (512, 8, 14, 14)

Sharding: batch (8) across the 8 NeuronCores, zero collectives.
Per-core device algorithm (all layouts partition-major, matmul compute bf16,
accumulation fp32 in PSUM):
    x_sb   [128, 2*1568]   c-chunk-major input (bf16)
    Q,Kp   [128, 4*1568]   per-head d-major Q (scale folded in W) and K + embT
    vT     [128, 13*512]   per-j-tile [n_tile, 4*128] V-transposed
    per (head, stripe of i):
      for jt in 13: S^T[j_tile, i_stripe] = Kp_tile^T @ Q_stripe (PE)
                    E = exp(S^T)                                  (ACT, PSUM->SBUF)
                    l[1,i] += ones^T @ E ; U^T[d,i] += vT_tile^T @ E   (PE, PSUM acc)
      recip = exp(-ln(l)) (ACT) ; bcast to 128 partitions (PE, K=1 matmul)
      out = U^T * bcast(recip) (DVE) -> DMA to out[h*128:(h+1)*128, i_stripe]
"""

import numpy as np
import ml_dtypes

# --- hardcoded problem shapes (self-contained: no spec.json / reference.py) ---
B = 8
C = 256          # input channels
F, HH, WW = 8, 14, 14
N = F * HH * WW  # 1568 positions
HEADS = 4
D = 128          # head dim
SCALE = D ** -0.5
N_CORES = 8

CC = C // 128            # c chunks (2)
NJ = (N + 127) // 128    # j tiles (13; last is 32 wide)
STRIPES = [(0, 1024), (1024, 544)]   # i stripes (psum-bank pairs)


def _chunks(start, width, bank=512):
    """Split [start, start+width) into psum-bank-aligned chunks (<=512 each)."""
    out = []
    c = 0
    while c < width:
        w = min(bank - ((start + c) % bank), width - c)
        out.append((c, w))
        c += w
    return out


_CACHE = {}


def _build():
    if "nc" in _CACHE:
        return _CACHE["nc"]

    import concourse.bacc as bacc
    import concourse.tile as tile
    import concourse.mybir as mybir

    f32 = mybir.dt.float32
    bf16 = mybir.dt.bfloat16
    AF = mybir.ActivationFunctionType

    nc = bacc.Bacc("TRN2", target_bir_lowering=False, debug=False,
                   enable_asserts=False, num_devices=N_CORES)

    x_d = nc.declare_dram_parameter("x", [128, CC * N], bf16, isOutput=False)
    wqk_d = nc.declare_dram_parameter("wqk", [128, CC * 1024], bf16, isOutput=False)
    wv_d = nc.declare_dram_parameter("wv", [128, CC * 512], bf16, isOutput=False)
    emb_d = nc.declare_dram_parameter("emb", [128, N], f32, isOutput=False)
    out_d = nc.declare_dram_parameter("out", [HEADS * D, N], f32, isOutput=True)

    with tile.TileContext(nc) as tc:
        with (
            tc.tile_pool(name="const", bufs=1) as constp,
            tc.tile_pool(name="epool", bufs=6) as epool,
            tc.tile_pool(name="rows", bufs=2) as rowsp,
            tc.tile_pool(name="rrows", bufs=2) as rrowsp,
            tc.tile_pool(name="outp", bufs=3) as outp,
            tc.tile_pool(name="rbp", bufs=2) as rbp,
            tc.tile_pool(name="ps_s", bufs=2, space="PSUM") as ps_s,
            tc.tile_pool(name="ps_l", bufs=1, space="PSUM") as ps_l,
            tc.tile_pool(name="ps_u", bufs=1, space="PSUM") as ps_u,
        ):
            # ---- load inputs ----
            x_sb = constp.tile([128, CC * N], bf16, tag="x")
            nc.sync.dma_start(x_sb[:, :], x_d.ap())
            wv_sb = constp.tile([128, CC * 512], bf16, tag="wv")
            nc.sync.dma_start(wv_sb[:, :], wv_d.ap())
            wqk_sb = constp.tile([128, CC * 1024], bf16, tag="wqk")
            nc.sync.dma_start(wqk_sb[:, :], wqk_d.ap())
            emb_sb = constp.tile([128, N], f32, tag="emb")
            nc.sync.dma_start(emb_sb[:, :], emb_d.ap())

            ones_sb = constp.tile([128, 128], bf16, tag="ones")
            nc.vector.memset(ones_sb[:, :], 1.0)

            q_sb = constp.tile([128, HEADS * N], bf16, tag="q")
            kp_sb = constp.tile([128, HEADS * N], bf16, tag="kp")
            vt_sb = constp.tile([128, NJ * 512], bf16, tag="vt")

            # ---- projection: vT tiles ----
            for jt in range(NJ):
                jw = min(128, N - jt * 128)
                pv = ps_u.tile([128, 1024], f32, tag="ps_u")
                for cc in range(CC):
                    nc.tensor.matmul(
                        pv[0:jw, 0:512],
                        lhsT=x_sb[:, cc * N + jt * 128: cc * N + jt * 128 + jw],
                        rhs=wv_sb[:, cc * 512: cc * 512 + 512],
                        start=(cc == 0), stop=(cc == CC - 1),
                    )
                nc.scalar.copy(vt_sb[0:jw, jt * 512: jt * 512 + 512], pv[0:jw, 0:512])

            # ---- projection: K' (+emb) then Q, head-major order ----
            for h in range(HEADS):
                for ot, dest in ((HEADS + h, "k"), (h, "q")):
                    for s0, sw in STRIPES:
                        pq = ps_s.tile([128, 1024], f32, tag="ps_s")
                        for c0, cw in _chunks(s0, sw):
                            for cc in range(CC):
                                nc.tensor.matmul(
                                    pq[:, c0:c0 + cw],
                                    lhsT=wqk_sb[:, cc * 1024 + ot * 128: cc * 1024 + ot * 128 + 128],
                                    rhs=x_sb[:, cc * N + s0 + c0: cc * N + s0 + c0 + cw],
                                    start=(cc == 0), stop=(cc == CC - 1),
                                )
                        if dest == "q":
                            nc.scalar.copy(q_sb[:, h * N + s0: h * N + s0 + sw], pq[:, 0:sw])
                        else:
                            nc.vector.tensor_add(
                                kp_sb[:, h * N + s0: h * N + s0 + sw],
                                pq[:, 0:sw], emb_sb[:, s0: s0 + sw],
                            )

            # ---- attention ----
            for h in range(HEADS):
                for s0, sw in STRIPES:
                    cks = _chunks(s0, sw)
                    pl = ps_l.tile([1, 1024], f32, tag="ps_l")
                    pu = ps_u.tile([128, 1024], f32, tag="ps_u")
                    for jt in range(NJ):
                        jw = min(128, N - jt * 128)
                        ps = ps_s.tile([128, 1024], f32, tag="ps_s")
                        for c0, cw in cks:
                            nc.tensor.matmul(
                                ps[0:jw, c0:c0 + cw],
                                lhsT=kp_sb[:, h * N + jt * 128: h * N + jt * 128 + jw],
                                rhs=q_sb[:, h * N + s0 + c0: h * N + s0 + c0 + cw],
                                start=True, stop=True,
                            )
                        et = epool.tile([128, 1024], bf16, tag="e")
                        nc.scalar.activation(et[0:jw, 0:sw], ps[0:jw, 0:sw], AF.Exp)
                        for c0, cw in cks:
                            nc.tensor.matmul(
                                pl[0:1, c0:c0 + cw],
                                lhsT=ones_sb[0:jw, 0:1],
                                rhs=et[0:jw, c0:c0 + cw],
                                start=(jt == 0), stop=(jt == NJ - 1),
                            )
                            nc.tensor.matmul(
                                pu[:, c0:c0 + cw],
                                lhsT=vt_sb[0:jw, jt * 512 + h * 128: jt * 512 + h * 128 + 128],
                                rhs=et[0:jw, c0:c0 + cw],
                                start=(jt == 0), stop=(jt == NJ - 1),
                            )
                    # normalize: recip = exp(-ln(l)); broadcast over partitions
                    lnr = rowsp.tile([1, 1024], f32, tag="rows")
                    nc.scalar.activation(lnr[0:1, 0:sw], pl[0:1, 0:sw], AF.Ln)
                    rr = rrowsp.tile([1, 1024], bf16, tag="rrows")
                    nc.scalar.activation(rr[0:1, 0:sw], lnr[0:1, 0:sw], AF.Exp, scale=-1.0)
                    pb = ps_s.tile([128, 1024], f32, tag="ps_s")
                    for c0, cw in cks:
                        nc.tensor.matmul(
                            pb[:, c0:c0 + cw],
                            lhsT=ones_sb[0:1, 0:128],
                            rhs=rr[0:1, c0:c0 + cw],
                            start=True, stop=True,
                        )
                    rb = rbp.tile([128, 1024], f32, tag="rb")
                    nc.vector.tensor_copy(rb[:, 0:sw], pb[:, 0:sw])
                    ot_sb = outp.tile([128, 1024], f32, tag="o")
                    nc.vector.tensor_mul(ot_sb[:, 0:sw], pu[:, 0:sw], rb[:, 0:sw])
                    nc.sync.dma_start(out_d.ap()[h * D:(h + 1) * D, s0:s0 + sw], ot_sb[:, 0:sw])

    nc.compile()
    _CACHE["nc"] = nc
    return nc


def _prep_inputs(fmap, W_qkv, pos_f, pos_h, pos_w):
    """Host-side shard + layout prep. Returns per-core input maps."""
    bf = ml_dtypes.bfloat16
    inner = HEADS * D
    emb = (pos_f[:, None, None, :] + pos_h[None, :, None, :]
           + pos_w[None, None, :, :]).reshape(N, D).astype(np.float32)
    embT = np.ascontiguousarray(emb.T)                      # [128, N]

    WqkT = W_qkv[:2 * inner].T.copy().astype(np.float32)    # [256, 1024]
    WqkT[:, :inner] *= SCALE                                # fold q scale
    wqk_host = np.ascontiguousarray(
        WqkT.reshape(CC, 128, 2 * inner).transpose(1, 0, 2).reshape(128, CC * 1024)).astype(bf)
    WvT = W_qkv[2 * inner:].T.copy().astype(np.float32)     # [256, 512]
    wv_host = np.ascontiguousarray(
        WvT.reshape(CC, 128, 512).transpose(1, 0, 2).reshape(128, CC * 512)).astype(bf)

    in_maps = []
    for b in range(B):
        xb = fmap[b].reshape(C, N).astype(np.float32)       # [256, N]
        x_host = np.ascontiguousarray(
            xb.reshape(CC, 128, N).transpose(1, 0, 2).reshape(128, CC * N)).astype(bf)
        in_maps.append({"x": x_host, "wqk": wqk_host, "wv": wv_host, "emb": embT})
    return in_maps


def kernel(fmap, W_qkv, pos_f, pos_h, pos_w):
    from concourse.bass_utils import run_bass_kernel_spmd

    nc = _build()
    in_maps = _prep_inputs(np.asarray(fmap), np.asarray(W_qkv), np.asarray(pos_f),
                           np.asarray(pos_h), np.asarray(pos_w))
    res = run_bass_kernel_spmd(nc, in_maps, core_ids=list(range(N_CORES)))
    out = np.stack([res.results[b]["out"].reshape(HEADS * D, F, HH, WW)
                    for b in range(B)])
    return out.astype(np.float32)


def benchmark(n_iters=30):
    """Estimate per-execution device time by timing n_iters async dispatches."""
    import time
    import jax
    from jax.sharding import Mesh, PartitionSpec
    from jax.experimental.shard_map import shard_map
    import concourse.mybir as mybir
    from concourse import bass2jax

    nc = _build()
    rng = np.random.default_rng(0)
    fmap = rng.standard_normal((B, C, F, HH, WW), dtype=np.float32)
    W = rng.standard_normal((3 * HEADS * D, C), dtype=np.float32) * C ** -0.5
    pf = rng.standard_normal((F, D), dtype=np.float32)
    ph = rng.standard_normal((HH, D), dtype=np.float32)
    pw = rng.standard_normal((WW, D), dtype=np.float32)
    in_maps = _prep_inputs(fmap, W, pf, ph, pw)

    bass2jax.install_neuronx_cc_hook()
    partition_name = nc.partition_id_tensor.name if nc.partition_id_tensor else None
    in_names, out_names, out_avals = [], [], []
    for alloc in nc.m.functions[0].allocations:
        if not isinstance(alloc, mybir.MemoryLocationSet):
            continue
        name = alloc.memorylocations[0].name
        if alloc.kind == "ExternalInput":
            if name != partition_name:
                in_names.append(name)
        elif alloc.kind == "ExternalOutput":
            out_names.append(name)
            out_avals.append(jax.core.ShapedArray(
                tuple(alloc.tensor_shape), mybir.dt.np(alloc.dtype)))
    n_params = len(in_names)
    zero_outs = [np.zeros(a.shape, a.dtype) for a in out_avals]
    all_in_names = in_names + out_names
    if partition_name is not None:
        all_in_names = all_in_names + [partition_name]

    def _body(*args):
        operands = list(args)
        if partition_name is not None:
            operands.append(bass2jax.partition_id_tensor())
        outs = bass2jax._bass_exec_p.bind(
            *operands, out_avals=tuple(out_avals), in_names=tuple(all_in_names),
            out_names=tuple(out_names), lowering_input_output_aliases=(),
            sim_require_finite=True, sim_require_nnan=True, nc=nc)
        return tuple(outs)

    devices = jax.devices()[:N_CORES]
    mesh = Mesh(np.asarray(devices), ("core",))
    specs = (PartitionSpec("core"),) * (n_params + len(out_names))
    fn = jax.jit(shard_map(_body, mesh=mesh, in_specs=specs,
                           out_specs=(PartitionSpec("core"),) * len(out_names),
                           check_rep=False))
    concat_in = [np.concatenate([in_maps[c][k] for c in range(N_CORES)], axis=0)
                 for k in in_names]
    concat_zero = [np.zeros((N_CORES * z.shape[0], *z.shape[1:]), z.dtype)
                   for z in zero_outs]
    args = [jax.device_put(a) for a in concat_in + concat_zero]

    outs = fn(*args)
    jax.block_until_ready(outs)
    # timed loop: async dispatch, single final block
    t0 = time.perf_counter()
    for _ in range(n_iters):
        outs = fn(*args)
    jax.block_until_ready(outs)
    t1 = time.perf_counter()
    per_run_ns = (t1 - t0) / n_iters * 1e9
    return per_run_ns


if __name__ == "__main__":
    ns = benchmark()
    print(f"HW exec time: {ns:.0f} ns")


# revision 29
# speedup vs baseline: 45.0167x; 45.0167x over previous
"""Trainium2 Bass kernel: 3D factorized-position attention (dense_transformer).

Reference computation (per batch b of 8):
    x = fmap[b].reshape(256, 1568)                       # channels x positions
    qkv = W_qkv @ x ; q,k,v heads of dim 128, 4 heads
    emb[n,128] = pos_f+pos_h+pos_w broadcast-sum
    sim = (q*scale) @ k^T + (q*scale) @ emb^T  ==  qs @ (k+embT)^T
    out = softmax(sim) @ v, reassembled to (512, 8, 14, 14)

Sharding: batch (8) across the 8 NeuronCores, zero collectives.
Per-core device algorithm (all layouts partition-major, matmul compute bf16,
accumulation fp32 in PSUM):
    x_sb   [128, 2*1568]   c-chunk-major input (bf16)
    Q,Kp   [128, 4*1568]   per-head d-major Q (scale folded in W) and K + embT
    vT     [128, 13*512]   per-j-tile [n_tile, 4*128] V-transposed
    per (head, stripe of i):
      for jt in 13: S^T[j_tile, i_stripe] = Kp_tile^T @ Q_stripe (PE)
                    E = exp(S^T)                                  (ACT, PSUM->SBUF)
                    l[1,i] += ones^T @ E ; U^T[d,i] += vT_tile^T @ E   (PE, PSUM acc)
      recip = exp(-ln(l)) (ACT) ; bcast to 128 partitions (PE, K=1 matmul)
      out = U^T * bcast(recip) (DVE) -> DMA to out[h*128:(h+1)*128, i_stripe]
"""

import numpy as np
import ml_dtypes

# --- hardcoded problem shapes (self-contained: no spec.json / reference.py) ---
B = 8
C = 256          # input channels
F, HH, WW = 8, 14, 14
N = F * HH * WW  # 1568 positions
HEADS = 4
D = 128          # head dim
SCALE = D ** -0.5
N_CORES = 8

CC = C // 128            # c chunks (2)
NJ = (N + 127) // 128    # j tiles (13; last is 32 wide)
STRIPES = [(0, 1024), (1024, 544)]   # i stripes (psum-bank pairs)


def _chunks(start, width, bank=512):
    """Split [start, start+width) into psum-bank-aligned chunks (<=512 each)."""
    out = []
    c = 0
    while c < width:
        w = min(bank - ((start + c) % bank), width - c)
        out.append((c, w))
        c += w
    return out


_CACHE = {}
INST_LABELS = {}


def _lab(bi, label):
    INST_LABELS[bi.ins.name] = label
    return bi


def _build(reps=1):
    key = ("nc", reps)
    if key in _CACHE:
        return _CACHE[key]

    import concourse.bacc as bacc
    import concourse.tile as tile
    import concourse.mybir as mybir

    f32 = mybir.dt.float32
    bf16 = mybir.dt.bfloat16
    AF = mybir.ActivationFunctionType

    from concourse.tile import add_dep_helper

    nc = bacc.Bacc("TRN2", target_bir_lowering=False, debug=False,
                   enable_asserts=False, num_devices=N_CORES)

    x_d = nc.declare_dram_parameter("x", [128, CC * N], bf16, isOutput=False)
    wqk_d = nc.declare_dram_parameter("wqk", [128, CC * 1024], bf16, isOutput=False)
    wv_d = nc.declare_dram_parameter("wv", [128, CC * 512], bf16, isOutput=False)
    emb_d = nc.declare_dram_parameter("emb", [128, N], f32, isOutput=False)
    out_d = nc.declare_dram_parameter("out", [HEADS * D, N], f32, isOutput=True)

    import contextlib
    with tile.TileContext(nc) as tc:
        rep_loop = tc.For_i(0, reps, 1) if reps > 1 else contextlib.nullcontext()
        with rep_loop:
          with (
            tc.tile_pool(name="const", bufs=1) as constp,
            tc.tile_pool(name="epool", bufs=6) as epool,
            tc.tile_pool(name="pairp", bufs=3) as pairp,
            tc.tile_pool(name="rows", bufs=2) as rowsp,
            tc.tile_pool(name="outp", bufs=3) as outp,
            tc.tile_pool(name="rbp", bufs=2) as rbp,
            tc.tile_pool(name="ps_s", bufs=2, space="PSUM") as ps_s,
            tc.tile_pool(name="ps_l", bufs=1, space="PSUM") as ps_l,
            tc.tile_pool(name="ps_u", bufs=1, space="PSUM") as ps_u,
        ):
            # ---- load inputs (c-chunk interleaved so compute starts early) ----
            x_sbs, wv_sbs = [], []
            for cc in range(CC):
                xt = constp.tile([128, N], bf16, tag=f"x{cc}", name=f"x{cc}")
                nc.sync.dma_start(xt[:, :], x_d.ap()[:, cc * N:(cc + 1) * N])
                x_sbs.append(xt)
                wt = constp.tile([128, 512], bf16, tag=f"wv{cc}", name=f"wv{cc}")
                nc.sync.dma_start(wt[:, :], wv_d.ap()[:, cc * 512:(cc + 1) * 512])
                wv_sbs.append(wt)
            wqk_sb = constp.tile([128, CC * 1024], bf16, tag="wqk")
            nc.sync.dma_start(wqk_sb[:, :], wqk_d.ap())
            emb_sb = constp.tile([128, N], f32, tag="emb")
            nc.sync.dma_start(emb_sb[:, :], emb_d.ap())

            ones_sb = constp.tile([128, 128], bf16, tag="ones")
            nc.vector.memset(ones_sb[:, :], 1.0)
            warm = rowsp.tile([1, 1024], bf16, tag="rows", name="warm")
            nc.scalar.activation(warm[0:1, 0:1], ones_sb[0:1, 0:1], AF.Exp)

            q_sb = constp.tile([128, HEADS * N], bf16, tag="q")
            kp_sb = constp.tile([128, HEADS * N], bf16, tag="kp")
            vt_sb = constp.tile([128, NJ * 512], bf16, tag="vt")

            # ---- projection: vT tiles ----
            for jt in range(NJ):
                jw = min(128, N - jt * 128)
                vpool = ps_u if jt % 2 == 0 else ps_l
                pv = vpool.tile([128, 512], f32, tag=vpool.name, name="pv")
                for cc in range(CC):
                    _lab(nc.tensor.matmul(
                        pv[0:jw, 0:512],
                        lhsT=x_sbs[cc][:, jt * 128: jt * 128 + jw],
                        rhs=wv_sbs[cc][:, 0:512],
                        start=(cc == 0), stop=(cc == CC - 1),
                    ), f"proj-vt{jt}")
                nc.scalar.copy(vt_sb[0:jw, jt * 512: jt * 512 + 512], pv[0:jw, 0:512])

            # ---- projection: K' (+emb) then Q, head-major order ----
            # Rotate psum pools (all free during projection) so several
            # projection groups can be in flight; epilogues split ACT/DVE.
            proj_pools = [ps_s, ps_s, ps_u]
            gidx = 0
            for h in range(HEADS):
                for ot, dest in ((HEADS + h, "k"), (h, "q")):
                    for s0, sw in STRIPES:
                        pool = proj_pools[gidx % len(proj_pools)]
                        gidx += 1
                        pq = pool.tile([128, 1024], f32, tag=pool.name, name="pq")
                        for c0, cw in _chunks(s0, sw):
                            for cc in range(CC):
                                last_proj_mm = _lab(nc.tensor.matmul(
                                    pq[:, c0:c0 + cw],
                                    lhsT=wqk_sb[:, cc * 1024 + ot * 128: cc * 1024 + ot * 128 + 128],
                                    rhs=x_sbs[cc][:, s0 + c0: s0 + c0 + cw],
                                    start=(cc == 0), stop=(cc == CC - 1),
                                ), f"proj-{dest}{h}s{s0}")
                        if dest == "q":
                            nc.scalar.copy(q_sb[:, h * N + s0: h * N + s0 + sw], pq[:, 0:sw])
                        else:
                            nc.vector.tensor_add(
                                kp_sb[:, h * N + s0: h * N + s0 + sw],
                                pq[:, 0:sw], emb_sb[:, s0: s0 + sw],
                            )

            def attn_mm(*args, label="attn", **kwargs):
                """Attention-phase matmul, ordered after the projection so it
                cannot starve projection of PSUM pool slots."""
                m = nc.tensor.matmul(*args, **kwargs)
                _lab(m, label)
                add_dep_helper(m.ins, last_proj_mm.ins, sync=True,
                               reason="attention PE work after projection")
                return m

            # ---- attention (software-pipelined) ----
            # Per (head, stripe): S^T matmul (PE) -> exp (ACT) -> l/U matmuls
            # (PE) with the E-consumers deferred one jt so ACT overlaps PE.
            # The normalize chain of stripe k is emitted inside stripe k+1's
            # pipeline so its DVE work hides under PE matmuls.
            def emit_consumers(st, jt, et):
                h, cks, pl, pu = st["h"], st["cks"], st["pl"], st["pu"]
                sw = st["sw"]
                jw = min(128, N - jt * 128)
                # U matmuls every jt; the ones-row l matmuls run on PAIRS of
                # E tiles pre-summed on DVE (linear in j), halving PE work.
                for ci, (c0, cw) in enumerate(cks):
                    attn_mm(
                        pu[:, c0:c0 + cw], label=f"u-mm h{h} jt{jt}",
                        lhsT=vt_sb[0:jw, jt * 512 + h * 128: jt * 512 + h * 128 + 128],
                        rhs=et[0:jw, c0:c0 + cw],
                        start=(jt == 0), stop=(jt == NJ - 1),
                    )
                lrhs = None
                if jt % 2 == 1:
                    e2 = pairp.tile([128, 1024], bf16, tag="pair", name="e2")
                    nc.vector.tensor_add(e2[:, 0:sw], st["pend_e"][:, 0:sw],
                                         et[:, 0:sw])
                    st["pend_e"] = None
                    lrhs = e2
                elif jt == NJ - 1:
                    lrhs = et
                else:
                    st["pend_e"] = et
                if lrhs is not None:
                    # ones stationary with M=128 broadcasts l to every PSUM
                    # partition at the same stream cost as M=1, so no separate
                    # broadcast step is needed at normalize time.
                    ljw = jw if jt == NJ - 1 else 128
                    for ci, (c0, cw) in enumerate(cks):
                        attn_mm(
                            pl[:, c0:c0 + cw], label=f"l-mm h{h} jt{jt}",
                            lhsT=ones_sb[0:ljw, 0:128],
                            rhs=lrhs[0:ljw, c0:c0 + cw],
                            start=(jt == 1), stop=(jt == NJ - 1),
                        )

            def emit_normalize(h, s0, sw, cks, pl, pu):
                # pl already holds l broadcast across partitions: reciprocal
                # then multiply-evacuate, chunked so DVE/DMA pipeline
                for ci, (c0, cw) in enumerate(cks):
                    rb = rbp.tile([128, 1024], f32, tag="rb", name="rb")
                    nc.vector.reciprocal_approx_fast(rb[:, 0:cw], pl[:, c0:c0 + cw])
                    ot_sb = outp.tile([128, 1024], f32, tag="o", name="ot_sb")
                    nc.vector.tensor_mul(ot_sb[:, 0:cw], pu[:, c0:c0 + cw], rb[:, 0:cw])
                    nc.sync.dma_start(
                        out_d.ap()[h * D:(h + 1) * D, s0 + c0:s0 + c0 + cw],
                        ot_sb[:, 0:cw])

            # Flatten all (head, stripe, jt) units into one stream; emit
            # S(k)+exp(k) at step k and the E-consumers of step k-DEPTH, so
            # the PE always has S work to do while ACT runs exp. A stripe's
            # normalize chain is emitted right after its last consumers
            # (which land inside the next stripe's early units).
            DEPTH = 2
            units = []
            for h in range(HEADS):
                for s0, sw in STRIPES:
                    st = {"h": h, "s0": s0, "sw": sw, "cks": _chunks(s0, sw),
                          "pl": None, "pu": None}
                    for jt in range(NJ):
                        units.append((st, jt))

            def emit_unit_s(st, jt):
                h, s0, sw = st["h"], st["s0"], st["sw"]
                if st["pl"] is None:
                    st["pl"] = ps_l.tile([128, 1024], f32, tag="ps_l", name="pl")
                    st["pu"] = ps_u.tile([128, 1024], f32, tag="ps_u", name="pu")
                jw = min(128, N - jt * 128)
                ps = ps_s.tile([128, 1024], f32, tag="ps_s")
                for c0, cw in st["cks"]:
                    attn_mm(
                        ps[0:jw, c0:c0 + cw], label=f"s-mm h{h} s{s0} jt{jt}",
                        lhsT=kp_sb[:, h * N + jt * 128: h * N + jt * 128 + jw],
                        rhs=q_sb[:, h * N + s0 + c0: h * N + s0 + c0 + cw],
                        start=True, stop=True,
                    )
                et = epool.tile([128, 1024], bf16, tag="e")
                nc.scalar.activation(et[0:jw, 0:sw], ps[0:jw, 0:sw], AF.Exp)
                return et

            ets = {}
            norm_due = {}
            for k in range(len(units) + DEPTH + 2):
                if k < len(units):
                    st, jt = units[k]
                    ets[k] = (st, jt, emit_unit_s(st, jt))
                if k in norm_due:
                    st = norm_due.pop(k)
                    emit_normalize(st["h"], st["s0"], st["sw"], st["cks"],
                                   st["pl"], st["pu"])
                j = k - DEPTH
                if 0 <= j < len(units):
                    st, jt, et = ets.pop(j)
                    emit_consumers(st, jt, et)
                    if jt == NJ - 1:
                        norm_due[k + 2] = st

    nc.compile()
    _CACHE[key] = nc
    return nc


def _prep_inputs(fmap, W_qkv, pos_f, pos_h, pos_w):
    """Host-side shard + layout prep. Returns per-core input maps."""
    bf = ml_dtypes.bfloat16
    inner = HEADS * D
    emb = (pos_f[:, None, None, :] + pos_h[None, :, None, :]
           + pos_w[None, None, :, :]).reshape(N, D).astype(np.float32)
    embT = np.ascontiguousarray(emb.T)                      # [128, N]

    WqkT = W_qkv[:2 * inner].T.copy().astype(np.float32)    # [256, 1024]
    WqkT[:, :inner] *= SCALE                                # fold q scale
    wqk_host = np.ascontiguousarray(
        WqkT.reshape(CC, 128, 2 * inner).transpose(1, 0, 2).reshape(128, CC * 1024)).astype(bf)
    WvT = W_qkv[2 * inner:].T.copy().astype(np.float32)     # [256, 512]
    wv_host = np.ascontiguousarray(
        WvT.reshape(CC, 128, 512).transpose(1, 0, 2).reshape(128, CC * 512)).astype(bf)

    in_maps = []
    for b in range(B):
        xb = fmap[b].reshape(C, N).astype(np.float32)       # [256, N]
        x_host = np.ascontiguousarray(
            xb.reshape(CC, 128, N).transpose(1, 0, 2).reshape(128, CC * N)).astype(bf)
        in_maps.append({"x": x_host, "wqk": wqk_host, "wv": wv_host, "emb": embT})
    return in_maps


def kernel(fmap, W_qkv, pos_f, pos_h, pos_w):
    from concourse.bass_utils import run_bass_kernel_spmd

    nc = _build()
    in_maps = _prep_inputs(np.asarray(fmap), np.asarray(W_qkv), np.asarray(pos_f),
                           np.asarray(pos_h), np.asarray(pos_w))
    res = run_bass_kernel_spmd(nc, in_maps, core_ids=list(range(N_CORES)))
    out = np.stack([res.results[b]["out"].reshape(HEADS * D, F, HH, WW)
                    for b in range(B)])
    return out.astype(np.float32)


def benchmark(n_iters=30, reps=1):
    """Estimate per-execution device time by timing n_iters async dispatches.
    With reps>1 the NEFF repeats the whole kernel body reps times in a
    hardware For_i loop, amortizing dispatch overhead for timing."""
    import time
    import jax
    from jax.sharding import Mesh, PartitionSpec
    from jax.experimental.shard_map import shard_map
    import concourse.mybir as mybir
    from concourse import bass2jax

    nc = _build(reps)
    rng = np.random.default_rng(0)
    fmap = rng.standard_normal((B, C, F, HH, WW), dtype=np.float32)
    W = rng.standard_normal((3 * HEADS * D, C), dtype=np.float32) * C ** -0.5
    pf = rng.standard_normal((F, D), dtype=np.float32)
    ph = rng.standard_normal((HH, D), dtype=np.float32)
    pw = rng.standard_normal((WW, D), dtype=np.float32)
    in_maps = _prep_inputs(fmap, W, pf, ph, pw)

    bass2jax.install_neuronx_cc_hook()
    partition_name = nc.partition_id_tensor.name if nc.partition_id_tensor else None
    in_names, out_names, out_avals = [], [], []
    for alloc in nc.m.functions[0].allocations:
        if not isinstance(alloc, mybir.MemoryLocationSet):
            continue
        name = alloc.memorylocations[0].name
        if alloc.kind == "ExternalInput":
            if name != partition_name:
                in_names.append(name)
        elif alloc.kind == "ExternalOutput":
            out_names.append(name)
            out_avals.append(jax.core.ShapedArray(
                tuple(alloc.tensor_shape), mybir.dt.np(alloc.dtype)))
    n_params = len(in_names)
    zero_outs = [np.zeros(a.shape, a.dtype) for a in out_avals]
    all_in_names = in_names + out_names
    if partition_name is not None:
        all_in_names = all_in_names + [partition_name]

    def _body(*args):
        operands = list(args)
        if partition_name is not None:
            operands.append(bass2jax.partition_id_tensor())
        outs = bass2jax._bass_exec_p.bind(
            *operands, out_avals=tuple(out_avals), in_names=tuple(all_in_names),
            out_names=tuple(out_names), lowering_input_output_aliases=(),
            sim_require_finite=True, sim_require_nnan=True, nc=nc)
        return tuple(outs)

    devices = jax.devices()[:N_CORES]
    mesh = Mesh(np.asarray(devices), ("core",))
    specs = (PartitionSpec("core"),) * (n_params + len(out_names))
    fn = jax.jit(shard_map(_body, mesh=mesh, in_specs=specs,
                           out_specs=(PartitionSpec("core"),) * len(out_names),
                           check_rep=False))
    concat_in = [np.concatenate([in_maps[c][k] for c in range(N_CORES)], axis=0)
                 for k in in_names]
    concat_zero = [np.zeros((N_CORES * z.shape[0], *z.shape[1:]), z.dtype)
                   for z in zero_outs]
    args = [jax.device_put(a) for a in concat_in + concat_zero]

    outs = fn(*args)
    jax.block_until_ready(outs)
    # timed loop: async dispatch, single final block
    t0 = time.perf_counter()
    for _ in range(n_iters):
        outs = fn(*args)
    jax.block_until_ready(outs)
    t1 = time.perf_counter()
    per_run_ns = (t1 - t0) / n_iters * 1e9
    return per_run_ns


if __name__ == "__main__":
    ns = benchmark()
    print(f"HW exec time: {ns:.0f} ns")
